# revision 20
# baseline (speedup 1.0000x reference)
"""MoE (8 experts, top-2) Trainium2 kernel.

Strategy: expert-parallel across the 8 NeuronCores. The gate (a
4096x1024 @ 1024x8 matmul + top-2 + renormalize, ~0.03% of total FLOPs)
is computed on the host in float64; it only produces routing metadata
(per-token expert ids + combine weights). Each core then runs the full
expert MLP for its expert's tokens:

    y_e = relu(x_e @ W1[e] + b1[e]) @ W2[e] + b2[e]

entirely on device in a fused Bass/Tile kernel (both matmuls, relu and
biases). The host scatters  out[t] = sum_e w_te * y_e[t]  back (the
combine weights are zero for non-selected experts, so routed compute is
mathematically identical to the reference's dense compute).

Device layout (per core, transposed activations so biases are
per-partition):
  MM1:  hT[FF, C] = W1.T @ xT   (+b1, relu)   lhsT = W1 k-tiles
  MM2:  yT[H, C]  = W2.T @ hT   (+b2)         lhsT = W2 k-tiles
with C = per-expert token capacity (padded), all accumulation in fp32
PSUM. Weights are streamed through small SBUF pools; xT and hT stay
SBUF-resident.

v7 head/tail optimizations (trace-driven; baseline lost ~22us to a cold
start; the head is CHIP-HBM-bound — all 8 cores land x+w1 through the
shared fabric at ~2.2TB/s — so the fix is giving the PE real work that
depends on only part of x):
  - MM1 runs in two k-phases: phase A accumulates kh0-3 into fp32
    SBUF partials (DVE eviction), phase B accumulates kh4-7 in PSUM,
    then one DVE scalar_tensor_tensor merges (psum + partial) and ACT
    applies relu+bias. Phase A is ~30us of PE work gated on only HALF
    of x (+ the small w1a tiles), so the PE saturates ~5us earlier.
  - x rides the SP ring in kh order (kh0-3 first); w1a/w1b/b on the
    ACT ring; kh4-7 are only needed ~30us later.
  - PE warmup: dummy matmuls on a zeroed scratch tile before and
    interleaved into the first chain (a PE idle gap re-throttles the
    HAM clock-gate back to 1.2GHz).
  - y is returned as f16 (combine is done on host in f32; quantizing y
    adds ~1e-4 rel err).
"""

import numpy as np

# ---------------------------------------------------------------- config
NUM_EXPERTS = 8
TOP_K = 2
B, S, H = 4, 1024, 1024
FF = 2 * H
T = B * S
P = 128
KH = H // P    # 8 k-tiles over H
KF = FF // P   # 16 k-tiles over FF
CAP_ALIGN = 4  # token capacity alignment (moving dim is arbitrary)
NTILE = 512    # max matmul moving free dim (one PSUM bank of fp32)
MM_DT_NAME = "f16"  # one of: bf16, f16, f32r, f32
N_WARM = 8     # PE warmup matmuls (HAM clock-gate ramp cover)

PROFILE = False       # set True (from test.py) to trace + record HW time
LAST_EXEC_NS = None
LAST_RESULTS = None

_cache = {}


def _mm_dt():
    import concourse.mybir as mybir
    import ml_dtypes

    return {
        "bf16": (mybir.dt.bfloat16, ml_dtypes.bfloat16),
        "f16": (mybir.dt.float16, np.float16),
        "f32r": (mybir.dt.float32r, np.float32),
        "f32": (mybir.dt.float32, np.float32),
    }[MM_DT_NAME]


def _build(CAP):
    """Build + compile the per-core Bass program (same for all cores)."""
    import concourse.bass as bass
    import concourse.mybir as mybir
    import concourse.tile as tile
    from concourse import bacc

    mm_dt, _ = _mm_dt()
    f32 = mybir.dt.float32
    ts, ds = bass.ts, bass.ds

    nc = bacc.Bacc("TRN2", debug=False, num_devices=NUM_EXPERTS)

    xt_d = nc.dram_tensor("xt_d", [KH, P, CAP], mm_dt, kind="ExternalInput")
    w1_d = nc.dram_tensor("w1_d", [KF, P, KH * P], mm_dt, kind="ExternalInput")
    b1_d = nc.dram_tensor("b1_d", [P, KF], f32, kind="ExternalInput")
    w2_d = nc.dram_tensor("w2_d", [KH, P, KF * P], mm_dt, kind="ExternalInput")
    b2_d = nc.dram_tensor("b2_d", [P, KH], f32, kind="ExternalInput")
    y_d = nc.dram_tensor("y_d", [KH, P, CAP], mm_dt, kind="ExternalOutput")

    # Equal n-tiles: a matmul's weight load (~97ns fp16) hides under the
    # moving-operand stream only if N is large enough; 3x384 beats
    # 512+512+128 (where the N=128 matmuls are weight-load-bound).
    n_splits = -(-CAP // NTILE)
    base = CAP // n_splits
    rem = CAP - base * n_splits
    n_tiles = []
    n0 = 0
    for j in range(n_splits):
        nsz = base + (1 if j < rem else 0)
        n_tiles.append((n0, nsz))
        n0 += nsz

    KH2 = KH // 2
    with tile.TileContext(nc) as tc:
        with (
            tc.tile_pool(name="const", bufs=1) as const,
            tc.tile_pool(name="warm", bufs=1) as warmp,
            tc.tile_pool(name="xtp", bufs=1) as xtp,
            tc.tile_pool(name="hp", bufs=1) as hp,
            tc.tile_pool(name="prt", bufs=1) as prt,
            tc.tile_pool(name="tmp", bufs=4) as tmpp,
            tc.tile_pool(name="w1ap", bufs=6) as w1ap,
            tc.tile_pool(name="w1bp", bufs=6) as w1bp,
            tc.tile_pool(name="w2p", bufs=3) as w2p,
            tc.tile_pool(name="yp", bufs=3) as yp,
            tc.tile_pool(name="psp", bufs=7, space="PSUM") as psp,
            tc.tile_pool(name="wps", bufs=1, space="PSUM") as wps,
        ):
            # ---- PE warmup: no input deps, issues immediately ----
            # (memset on the otherwise-idle GpSimd: its sequencer is
            # ready ~1.4us before DVE, so warmups start earlier)
            warm = warmp.tile([P, NTILE], mm_dt)
            nc.gpsimd.memset(warm[:], 0.0)
            wacc = wps.tile([P, NTILE], f32)

            def warmup(n):
                for _ in range(n):
                    nc.tensor.matmul(
                        wacc[:],
                        warm[:, 0:P],
                        warm[:],
                        start=True,
                        stop=True,
                        skip_group_check=True,
                    )

            warmup(N_WARM)

            w1a_tiles = {}
            w1b_tiles = {}

            def load_w1a(kf):
                w1a = w1ap.tile([P, KH2 * P], mm_dt, tag="w1a", name="w1a")
                nc.scalar.dma_start(w1a[:], w1_d.ap()[kf, :, 0 : KH2 * P])
                w1a_tiles[kf] = w1a

            def load_w1b(kf):
                w1b = w1bp.tile([P, KH2 * P], mm_dt, tag="w1b", name="w1b")
                nc.scalar.dma_start(w1b[:], w1_d.ap()[kf, :, KH2 * P : KH * P])
                w1b_tiles[kf] = w1b

            # Head order: w1a[0] + x kh0-3 are all phase A's first chain
            # needs; everything else streams behind. kh0 is split so the
            # very first chain matmul only waits for a third of it.
            load_w1a(0)
            xts = [None] * KH
            NS0 = n_tiles[0][1]
            x0a = xtp.tile([P, NS0], mm_dt, tag="xt0a")
            nc.sync.dma_start(x0a[:], xt_d.ap()[0, :, 0:NS0])
            x0b = xtp.tile([P, CAP - NS0], mm_dt, tag="xt0b")
            nc.sync.dma_start(x0b[:], xt_d.ap()[0, :, NS0:CAP])
            for kh in range(1, KH):
                xk = xtp.tile([P, CAP], mm_dt, tag=f"xt{kh}")
                nc.sync.dma_start(xk[:], xt_d.ap()[kh])
                xts[kh] = xk
            load_w1a(1)
            load_w1a(2)

            def x_ap(kh, n0, nsz):
                if kh != 0:
                    return xts[kh][:, ds(n0, nsz)]
                if n0 + nsz <= NS0:
                    return x0a[:, ds(n0, nsz)]
                return x0b[:, ds(n0 - NS0, nsz)]

            b1t = const.tile([P, KF], f32)
            nc.scalar.dma_start(b1t[:], b1_d.ap())
            b2t = const.tile([P, KH], f32)
            nc.scalar.dma_start(b2t[:], b2_d.ap())

            h = hp.tile([P, KF, CAP], mm_dt)
            part = prt.tile([P, KF, CAP], f32)

            # warmup matmuls between first-chain MMs (a PE idle gap
            # re-throttles the HAM clock-gate back to 1.2GHz); lighter
            # insurance on the second chain for x2/x3 arrival jitter
            ILV_A = [3, 3, 3]
            ILV_A2 = [1, 1, 1]

            # ---- MM1 phase A: part[kf] = W1[kh0-3].T @ xT[kh0-3] ----
            def mm1_a(kf, j, interleave=None):
                n0, nsz = n_tiles[j]
                w1a = w1a_tiles[kf]
                acc = psp.tile([P, NTILE], f32)
                for i in range(KH2):
                    nc.tensor.matmul(
                        acc[:, :nsz],
                        w1a[:, ts(i, P)],
                        x_ap(i, n0, nsz),
                        start=(i == 0),
                        stop=(i == KH2 - 1),
                        skip_group_check=interleave is not None,
                    )
                    if interleave is not None and i < KH2 - 1:
                        warmup(interleave[i])
                # alternate eviction engine: keeps both DVE and ACT
                # under ~50% so PSUM banks recycle without PE stalls
                if (kf * len(n_tiles) + j) % 2 == 0:
                    nc.vector.tensor_scalar_add(
                        part[:, kf, ds(n0, nsz)], acc[:, :nsz], 0.0
                    )
                else:
                    nc.scalar.activation(
                        part[:, kf, ds(n0, nsz)],
                        acc[:, :nsz],
                        mybir.ActivationFunctionType.Identity,
                    )

            for kf in range(KF):
                if kf + 3 < KF and kf + 3 not in w1a_tiles:
                    load_w1a(kf + 3)
                for j in range(len(n_tiles)):
                    ilv = None
                    if kf == 0 and j == 0:
                        ilv = ILV_A
                    elif kf == 0 and j == 1:
                        ilv = ILV_A2
                    mm1_a(kf, j, interleave=ilv)
                if kf < 3:
                    load_w1b(kf)

            # ---- MM1 phase B: h[kf] = relu(part + W1[kh4-7].T @ x + b1) ----
            def mm1_b(kf, j):
                n0, nsz = n_tiles[j]
                w1b = w1b_tiles[kf]
                acc = psp.tile([P, NTILE], f32)
                for i in range(KH2):
                    nc.tensor.matmul(
                        acc[:, :nsz],
                        w1b[:, ts(i, P)],
                        xts[KH2 + i][:, ds(n0, nsz)],
                        start=(i == 0),
                        stop=(i == KH2 - 1),
                    )
                tm = tmpp.tile([P, NTILE], f32)
                nc.vector.scalar_tensor_tensor(
                    tm[:, :nsz],
                    acc[:, :nsz],
                    0.0,
                    part[:, kf, ds(n0, nsz)],
                    mybir.AluOpType.add,
                    mybir.AluOpType.add,
                )
                nc.scalar.activation(
                    h[:, kf, ds(n0, nsz)],
                    tm[:, :nsz],
                    mybir.ActivationFunctionType.Relu,
                    bias=b1t[:, kf : kf + 1],
                )

            for kf in range(KF):
                if kf + 3 < KF and kf + 3 not in w1b_tiles:
                    load_w1b(kf + 3)
                for j in range(len(n_tiles)):
                    mm1_b(kf, j)

            # ---- MM2: yT[m, :] = W2.T @ hT + b2 ----
            def mm2_chunk(m, n0, nsz):
                acc = psp.tile([P, NTILE], f32)
                for k in range(KF):
                    nc.tensor.matmul(
                        acc[:, :nsz],
                        w2_tiles[m][:, ts(k, P)],
                        h[:, k, ds(n0, nsz)],
                        start=(k == 0),
                        stop=(k == KF - 1),
                    )
                yt = yp.tile([P, NTILE], mm_dt)
                nc.scalar.activation(
                    yt[:, :nsz],
                    acc[:, :nsz],
                    mybir.ActivationFunctionType.Identity,
                    bias=b2t[:, m : m + 1],
                )
                nc.sync.dma_start(y_d.ap()[m, :, ds(n0, nsz)], yt[:, :nsz])

            w2_tiles = {}
            for m in range(KH):
                w2t = w2p.tile([P, KF * P], mm_dt)
                nc.scalar.dma_start(w2t[:], w2_d.ap()[m])
                w2_tiles[m] = w2t
                for jj, (n0, nsz) in enumerate(n_tiles):
                    last = m == KH - 1 and jj == len(n_tiles) - 1
                    if last:
                        # split the final chunk so the first half's
                        # eviction+store overlaps the second half's
                        # matmul chain, shortening the critical tail
                        h1 = (nsz // 2 + 3) & ~3
                        mm2_chunk(m, n0, h1)
                        mm2_chunk(m, n0 + h1, nsz - h1)
                    else:
                        mm2_chunk(m, n0, nsz)

    nc.compile()
    return nc


def _install_profile_shim():
    """Make run_bass_kernel_spmd(trace=True) work under axon in this
    container (the boot-time antenv.axon_hooks install is absent)."""
    import contextlib
    import ctypes
    import sys
    import types

    if "antenv.axon_hooks" in sys.modules:
        return
    so_path = "/opt/axon/libaxon_pjrt.so"
    lib = ctypes.CDLL(so_path)
    if not hasattr(lib, "axon_start_nrt_profile"):
        return
    lib.axon_start_nrt_profile.argtypes = [
        ctypes.POINTER(ctypes.c_int64),
        ctypes.c_size_t,
    ]
    lib.axon_start_nrt_profile.restype = ctypes.c_int64
    lib.axon_stop_nrt_profile.argtypes = [ctypes.c_char_p]
    lib.axon_stop_nrt_profile.restype = ctypes.c_int64

    @contextlib.contextmanager
    def _hook(output_dir, device_ids):
        import jax

        jax.devices()
        if device_ids:
            ids = (ctypes.c_int64 * len(device_ids))(*device_ids)
            rc = lib.axon_start_nrt_profile(ids, len(device_ids))
        else:
            rc = lib.axon_start_nrt_profile(None, 0)
        if rc != 0:
            raise RuntimeError(f"axon_start_nrt_profile rc={rc}")
        try:
            yield
        finally:
            n = lib.axon_stop_nrt_profile(str(output_dir).encode())
            print(f"ntff profile: {n} file(s) in {output_dir}", file=sys.stderr)

    mod = types.ModuleType("antenv.axon_hooks")
    mod.get_axon_ntff_profile_hook = lambda: _hook
    mod.set_axon_ntff_profile_hook = lambda h: None
    sys.modules["antenv.axon_hooks"] = mod

    import concourse.bass_utils as bu

    bu.upload_artifacts = lambda tmpdir: str(tmpdir)


# ---------------------------------------------------------------- host side

def _route(xf, Wg, bg):
    """Top-2 routing on host, float64 scoring. Returns (top2 [T,2] int,
    w [T,2] float32 renormalized combine weights)."""
    logits = xf.astype(np.float64) @ Wg.astype(np.float64) + bg.astype(np.float64)
    top2 = np.argsort(-logits, axis=-1, kind="stable")[:, :TOP_K]
    lv = np.take_along_axis(logits, top2, axis=1)
    lv = lv - lv.max(axis=1, keepdims=True)
    ev = np.exp(lv)
    w = ev / ev.sum(axis=1, keepdims=True)
    return top2, w.astype(np.float32)


def _prep_weights(W1, b1, W2, b2, np_dt):
    """Per-expert DRAM layouts for the device program."""
    per_expert = []
    for e in range(NUM_EXPERTS):
        w1g = (
            W1[e]
            .reshape(KH, P, KF, P)
            .transpose(2, 1, 0, 3)
            .reshape(KF, P, KH * P)
            .astype(np_dt)
        )
        w2g = (
            W2[e]
            .reshape(KF, P, KH, P)
            .transpose(2, 1, 0, 3)
            .reshape(KH, P, KF * P)
            .astype(np_dt)
        )
        b1g = np.ascontiguousarray(b1[e].reshape(KF, P).T).astype(np.float32)
        b2g = np.ascontiguousarray(b2[e].reshape(KH, P).T).astype(np.float32)
        per_expert.append((w1g, w2g, b1g, b2g))
    return per_expert


def kernel(x, Wg, bg, W1, b1, W2, b2):
    global LAST_EXEC_NS, LAST_RESULTS

    x = np.asarray(x, dtype=np.float32)
    Wg = np.asarray(Wg, dtype=np.float32)
    bg = np.asarray(bg, dtype=np.float32)
    W1 = np.asarray(W1, dtype=np.float32)
    b1 = np.asarray(b1, dtype=np.float32)
    W2 = np.asarray(W2, dtype=np.float32)
    b2 = np.asarray(b2, dtype=np.float32)

    _, np_dt = _mm_dt()
    if PROFILE:
        _install_profile_shim()

    from concourse.bass_utils import run_bass_kernel_spmd

    xf = x.reshape(T, H)
    top2, w = _route(xf, Wg, bg)

    per_expert = _prep_weights(W1, b1, W2, b2, np_dt)

    # token lists per expert
    idx_list = []
    wgt_list = []
    for e in range(NUM_EXPERTS):
        mask = top2 == e  # [T, 2]
        idx = np.where(mask.any(axis=1))[0]
        slot = mask[idx, 1].astype(np.int64)  # 0 if slot0, 1 if slot1
        idx_list.append(idx)
        wgt_list.append(w[idx, slot])

    out = np.zeros((T, H), dtype=np.float32)
    max_count = max(len(i) for i in idx_list)
    # capacity: fit the hottest expert exactly (aligned), bounded so a
    # pathological distribution falls back to multiple rounds
    CAP = min(2048, max(512, -(-max_count // CAP_ALIGN) * CAP_ALIGN))
    if CAP not in _cache:
        _cache[CAP] = _build(CAP)
    nc = _cache[CAP]
    n_rounds = max(1, -(-max_count // CAP))

    for r in range(n_rounds):
        in_maps = []
        chunk_idx = []
        for e in range(NUM_EXPERTS):
            idx = idx_list[e][r * CAP : (r + 1) * CAP]
            chunk_idx.append(idx)
            c = len(idx)
            xe = np.zeros((H, CAP), dtype=np_dt)
            if c:
                xe[:, :c] = xf[idx].T.astype(np_dt)
            w1g, w2g, b1g, b2g = per_expert[e]
            in_maps.append(
                {
                    "xt_d": xe.reshape(KH, P, CAP),
                    "w1_d": w1g,
                    "b1_d": b1g,
                    "w2_d": w2g,
                    "b2_d": b2g,
                }
            )
        res = run_bass_kernel_spmd(
            nc,
            in_maps,
            core_ids=list(range(NUM_EXPERTS)),
            trace=bool(PROFILE),
        )
        if PROFILE:
            LAST_EXEC_NS = res.exec_time_ns
            LAST_RESULTS = res
        for e in range(NUM_EXPERTS):
            idx = chunk_idx[e]
            c = len(idx)
            if not c:
                continue
            yT = res.results[e]["y_d"].reshape(H, CAP)  # [H, CAP]
            we = wgt_list[e][r * CAP : (r + 1) * CAP]
            out[idx] += we[:, None] * np.asarray(yT[:, :c].T, dtype=np.float32)

    return out.reshape(B, S, H)


# revision 25
# speedup vs baseline: 1.0184x; 1.0184x over previous
"""MoE (8 experts, top-2) Trainium2 kernel.

Strategy: expert-parallel across the 8 NeuronCores. The gate (a
4096x1024 @ 1024x8 matmul + top-2 + renormalize, ~0.03% of total FLOPs)
is computed on the host in float64; it only produces routing metadata
(per-token expert ids + combine weights). Each core then runs the full
expert MLP for its expert's tokens:

    y_e = relu(x_e @ W1[e] + b1[e]) @ W2[e] + b2[e]

entirely on device in a fused Bass/Tile kernel (both matmuls, relu and
biases). The host scatters  out[t] = sum_e w_te * y_e[t]  back (the
combine weights are zero for non-selected experts, so routed compute is
mathematically identical to the reference's dense compute).

Device layout (per core, transposed activations so biases are
per-partition):
  MM1:  hT[FF, C] = W1.T @ xT   (+b1, relu)   lhsT = W1 k-tiles
  MM2:  yT[H, C]  = W2.T @ hT   (+b2)         lhsT = W2 k-tiles
with C = per-expert token capacity (padded), all accumulation in fp32
PSUM. Weights are streamed through small SBUF pools; xT and hT stay
SBUF-resident.

v7 head/tail optimizations (trace-driven; baseline lost ~22us to a cold
start; the head is CHIP-HBM-bound — all 8 cores land x+w1 through the
shared fabric at ~2.2TB/s — so the fix is giving the PE real work that
depends on only part of x):
  - MM1 runs in two k-phases: phase A accumulates kh0-3 into fp32
    SBUF partials (DVE eviction), phase B accumulates kh4-7 in PSUM,
    then one DVE scalar_tensor_tensor merges (psum + partial) and ACT
    applies relu+bias. Phase A is ~30us of PE work gated on only HALF
    of x (+ the small w1a tiles), so the PE saturates ~5us earlier.
  - x rides the SP ring in kh order (kh0-3 first); w1a/w1b/b on the
    ACT ring; kh4-7 are only needed ~30us later.
  - PE warmup: dummy matmuls on a zeroed scratch tile before and
    interleaved into the first chain (a PE idle gap re-throttles the
    HAM clock-gate back to 1.2GHz).
  - y is returned as f16 (combine is done on host in f32; quantizing y
    adds ~1e-4 rel err).
"""

import numpy as np

# ---------------------------------------------------------------- config
NUM_EXPERTS = 8
TOP_K = 2
B, S, H = 4, 1024, 1024
FF = 2 * H
T = B * S
P = 128
KH = H // P    # 8 k-tiles over H
KF = FF // P   # 16 k-tiles over FF
CAP_ALIGN = 4  # token capacity alignment (moving dim is arbitrary)
NTILE = 512    # max matmul moving free dim (one PSUM bank of fp32)
MM_DT_NAME = "f16"  # one of: bf16, f16, f32r, f32
N_WARM = 8     # PE warmup matmuls (HAM clock-gate ramp cover)

PROFILE = False       # set True (from test.py) to trace + record HW time
LAST_EXEC_NS = None
LAST_RESULTS = None

_cache = {}


def _mm_dt():
    import concourse.mybir as mybir
    import ml_dtypes

    return {
        "bf16": (mybir.dt.bfloat16, ml_dtypes.bfloat16),
        "f16": (mybir.dt.float16, np.float16),
        "f32r": (mybir.dt.float32r, np.float32),
        "f32": (mybir.dt.float32, np.float32),
    }[MM_DT_NAME]


def _build(CAP):
    """Build + compile the per-core Bass program (same for all cores)."""
    import concourse.bass as bass
    import concourse.mybir as mybir
    import concourse.tile as tile
    from concourse import bacc

    mm_dt, _ = _mm_dt()
    f32 = mybir.dt.float32
    ts, ds = bass.ts, bass.ds

    nc = bacc.Bacc("TRN2", debug=False, num_devices=NUM_EXPERTS)

    xt_d = nc.dram_tensor("xt_d", [KH, P, CAP], mm_dt, kind="ExternalInput")
    w1_d = nc.dram_tensor("w1_d", [KF, P, KH * P], mm_dt, kind="ExternalInput")
    b1_d = nc.dram_tensor("b1_d", [P, KF], f32, kind="ExternalInput")
    w2_d = nc.dram_tensor("w2_d", [KH, P, KF * P], mm_dt, kind="ExternalInput")
    b2_d = nc.dram_tensor("b2_d", [P, KH], f32, kind="ExternalInput")
    y_d = nc.dram_tensor("y_d", [KH, P, CAP], mm_dt, kind="ExternalOutput")

    # Equal n-tiles: a matmul's weight load (~97ns fp16) hides under the
    # moving-operand stream only if N is large enough; 3x384 beats
    # 512+512+128 (where the N=128 matmuls are weight-load-bound).
    n_splits = -(-CAP // NTILE)
    base = CAP // n_splits
    rem = CAP - base * n_splits
    n_tiles = []
    n0 = 0
    for j in range(n_splits):
        nsz = base + (1 if j < rem else 0)
        n_tiles.append((n0, nsz))
        n0 += nsz

    KH2 = KH // 2
    with tile.TileContext(nc) as tc:
        with (
            tc.tile_pool(name="const", bufs=1) as const,
            tc.tile_pool(name="warm", bufs=1) as warmp,
            tc.tile_pool(name="xtp", bufs=1) as xtp,
            tc.tile_pool(name="hp", bufs=1) as hp,
            tc.tile_pool(name="prt", bufs=1) as prt,
            tc.tile_pool(name="tmp", bufs=4) as tmpp,
            tc.tile_pool(name="w1ap", bufs=6) as w1ap,
            tc.tile_pool(name="w1bp", bufs=6) as w1bp,
            tc.tile_pool(name="w2p", bufs=3) as w2p,
            tc.tile_pool(name="yp", bufs=3) as yp,
            tc.tile_pool(name="psp", bufs=7, space="PSUM") as psp,
            tc.tile_pool(name="wps", bufs=1, space="PSUM") as wps,
        ):
            # ---- PE warmup: no input deps, issues immediately ----
            warm = warmp.tile([P, NTILE], mm_dt)
            nc.vector.memset(warm[:], 0.0)
            wacc = wps.tile([P, NTILE], f32)

            def warmup(n):
                for _ in range(n):
                    nc.tensor.matmul(
                        wacc[:],
                        warm[:, 0:P],
                        warm[:],
                        start=True,
                        stop=True,
                        skip_group_check=True,
                    )

            warmup(N_WARM)

            w1a_tiles = {}
            w1b_tiles = {}

            def load_w1a(kf):
                w1a = w1ap.tile([P, KH2 * P], mm_dt, tag="w1a", name="w1a")
                nc.scalar.dma_start(w1a[:], w1_d.ap()[kf, :, 0 : KH2 * P])
                w1a_tiles[kf] = w1a

            def load_w1b(kf):
                w1b = w1bp.tile([P, KH2 * P], mm_dt, tag="w1b", name="w1b")
                nc.scalar.dma_start(w1b[:], w1_d.ap()[kf, :, KH2 * P : KH * P])
                w1b_tiles[kf] = w1b

            # Head order: w1a[0] + x kh0-3 are all phase A's first chain
            # needs; everything else streams behind.
            load_w1a(0)
            xts = [None] * KH
            for kh in range(KH):
                xk = xtp.tile([P, CAP], mm_dt, tag=f"xt{kh}")
                nc.sync.dma_start(xk[:], xt_d.ap()[kh])
                xts[kh] = xk
            load_w1a(1)
            load_w1a(2)

            b1t = const.tile([P, KF], f32)
            nc.scalar.dma_start(b1t[:], b1_d.ap())
            b2t = const.tile([P, KH], f32)
            nc.scalar.dma_start(b2t[:], b2_d.ap())

            h = hp.tile([P, KF, CAP], mm_dt)
            part = prt.tile([P, KF, CAP], f32)

            # warmup matmuls between first-chain MMs (a PE idle gap
            # re-throttles the HAM clock-gate back to 1.2GHz)
            ILV_A = [3, 3, 3]

            # ---- MM1 phase A: part[kf] = W1[kh0-3].T @ xT[kh0-3] ----
            def mm1_a(kf, j, interleave=None):
                n0, nsz = n_tiles[j]
                w1a = w1a_tiles[kf]
                acc = psp.tile([P, NTILE], f32)
                for i in range(KH2):
                    nc.tensor.matmul(
                        acc[:, :nsz],
                        w1a[:, ts(i, P)],
                        xts[i][:, ds(n0, nsz)],
                        start=(i == 0),
                        stop=(i == KH2 - 1),
                        skip_group_check=interleave is not None,
                    )
                    if interleave is not None and i < KH2 - 1:
                        warmup(interleave[i])
                # alternate eviction engine: keeps both DVE and ACT
                # under ~50% so PSUM banks recycle without PE stalls
                if (kf * len(n_tiles) + j) % 2 == 0:
                    nc.vector.tensor_scalar_add(
                        part[:, kf, ds(n0, nsz)], acc[:, :nsz], 0.0
                    )
                else:
                    nc.scalar.activation(
                        part[:, kf, ds(n0, nsz)],
                        acc[:, :nsz],
                        mybir.ActivationFunctionType.Identity,
                    )

            for kf in range(KF):
                if kf + 3 < KF and kf + 3 not in w1a_tiles:
                    load_w1a(kf + 3)
                for j in range(len(n_tiles)):
                    mm1_a(kf, j, interleave=ILV_A if (kf == 0 and j == 0) else None)
                if kf < 3:
                    load_w1b(kf)

            # ---- MM1 phase B: h[kf] = relu(part + W1[kh4-7].T @ x + b1) ----
            def mm1_b(kf, j):
                n0, nsz = n_tiles[j]
                w1b = w1b_tiles[kf]
                acc = psp.tile([P, NTILE], f32)
                for i in range(KH2):
                    nc.tensor.matmul(
                        acc[:, :nsz],
                        w1b[:, ts(i, P)],
                        xts[KH2 + i][:, ds(n0, nsz)],
                        start=(i == 0),
                        stop=(i == KH2 - 1),
                    )
                tm = tmpp.tile([P, NTILE], f32)
                nc.vector.scalar_tensor_tensor(
                    tm[:, :nsz],
                    acc[:, :nsz],
                    0.0,
                    part[:, kf, ds(n0, nsz)],
                    mybir.AluOpType.add,
                    mybir.AluOpType.add,
                )
                nc.scalar.activation(
                    h[:, kf, ds(n0, nsz)],
                    tm[:, :nsz],
                    mybir.ActivationFunctionType.Relu,
                    bias=b1t[:, kf : kf + 1],
                )

            for kf in range(KF):
                if kf + 3 < KF and kf + 3 not in w1b_tiles:
                    load_w1b(kf + 3)
                for j in range(len(n_tiles)):
                    mm1_b(kf, j)

            # ---- MM2: yT[m, :] = W2.T @ hT + b2 ----
            def mm2_chunk(m, n0, nsz):
                acc = psp.tile([P, NTILE], f32)
                for k in range(KF):
                    nc.tensor.matmul(
                        acc[:, :nsz],
                        w2_tiles[m][:, ts(k, P)],
                        h[:, k, ds(n0, nsz)],
                        start=(k == 0),
                        stop=(k == KF - 1),
                    )
                yt = yp.tile([P, NTILE], mm_dt)
                nc.scalar.activation(
                    yt[:, :nsz],
                    acc[:, :nsz],
                    mybir.ActivationFunctionType.Identity,
                    bias=b2t[:, m : m + 1],
                )
                nc.sync.dma_start(y_d.ap()[m, :, ds(n0, nsz)], yt[:, :nsz])

            w2_tiles = {}
            for m in range(KH):
                w2t = w2p.tile([P, KF * P], mm_dt)
                nc.scalar.dma_start(w2t[:], w2_d.ap()[m])
                w2_tiles[m] = w2t
                for jj, (n0, nsz) in enumerate(n_tiles):
                    last = m == KH - 1 and jj == len(n_tiles) - 1
                    if last:
                        # split the final chunk so the first half's
                        # eviction+store overlaps the second half's
                        # matmul chain, shortening the critical tail
                        h1 = (nsz // 2 + 3) & ~3
                        mm2_chunk(m, n0, h1)
                        mm2_chunk(m, n0 + h1, nsz - h1)
                    else:
                        mm2_chunk(m, n0, nsz)

    nc.compile()
    return nc


def _install_profile_shim():
    """Make run_bass_kernel_spmd(trace=True) work under axon in this
    container (the boot-time antenv.axon_hooks install is absent)."""
    import contextlib
    import ctypes
    import sys
    import types

    if "antenv.axon_hooks" in sys.modules:
        return
    so_path = "/opt/axon/libaxon_pjrt.so"
    lib = ctypes.CDLL(so_path)
    if not hasattr(lib, "axon_start_nrt_profile"):
        return
    lib.axon_start_nrt_profile.argtypes = [
        ctypes.POINTER(ctypes.c_int64),
        ctypes.c_size_t,
    ]
    lib.axon_start_nrt_profile.restype = ctypes.c_int64
    lib.axon_stop_nrt_profile.argtypes = [ctypes.c_char_p]
    lib.axon_stop_nrt_profile.restype = ctypes.c_int64

    @contextlib.contextmanager
    def _hook(output_dir, device_ids):
        import jax

        jax.devices()
        if device_ids:
            ids = (ctypes.c_int64 * len(device_ids))(*device_ids)
            rc = lib.axon_start_nrt_profile(ids, len(device_ids))
        else:
            rc = lib.axon_start_nrt_profile(None, 0)
        if rc != 0:
            raise RuntimeError(f"axon_start_nrt_profile rc={rc}")
        try:
            yield
        finally:
            n = lib.axon_stop_nrt_profile(str(output_dir).encode())
            print(f"ntff profile: {n} file(s) in {output_dir}", file=sys.stderr)

    mod = types.ModuleType("antenv.axon_hooks")
    mod.get_axon_ntff_profile_hook = lambda: _hook
    mod.set_axon_ntff_profile_hook = lambda h: None
    sys.modules["antenv.axon_hooks"] = mod

    import concourse.bass_utils as bu

    bu.upload_artifacts = lambda tmpdir: str(tmpdir)


# ---------------------------------------------------------------- host side

def _route(xf, Wg, bg):
    """Top-2 routing on host, float64 scoring. Returns (top2 [T,2] int,
    w [T,2] float32 renormalized combine weights)."""
    logits = xf.astype(np.float64) @ Wg.astype(np.float64) + bg.astype(np.float64)
    top2 = np.argsort(-logits, axis=-1, kind="stable")[:, :TOP_K]
    lv = np.take_along_axis(logits, top2, axis=1)
    lv = lv - lv.max(axis=1, keepdims=True)
    ev = np.exp(lv)
    w = ev / ev.sum(axis=1, keepdims=True)
    return top2, w.astype(np.float32)


def _prep_weights(W1, b1, W2, b2, np_dt):
    """Per-expert DRAM layouts for the device program."""
    per_expert = []
    for e in range(NUM_EXPERTS):
        w1g = (
            W1[e]
            .reshape(KH, P, KF, P)
            .transpose(2, 1, 0, 3)
            .reshape(KF, P, KH * P)
            .astype(np_dt)
        )
        w2g = (
            W2[e]
            .reshape(KF, P, KH, P)
            .transpose(2, 1, 0, 3)
            .reshape(KH, P, KF * P)
            .astype(np_dt)
        )
        b1g = np.ascontiguousarray(b1[e].reshape(KF, P).T).astype(np.float32)
        b2g = np.ascontiguousarray(b2[e].reshape(KH, P).T).astype(np.float32)
        per_expert.append((w1g, w2g, b1g, b2g))
    return per_expert


def kernel(x, Wg, bg, W1, b1, W2, b2):
    global LAST_EXEC_NS, LAST_RESULTS

    x = np.asarray(x, dtype=np.float32)
    Wg = np.asarray(Wg, dtype=np.float32)
    bg = np.asarray(bg, dtype=np.float32)
    W1 = np.asarray(W1, dtype=np.float32)
    b1 = np.asarray(b1, dtype=np.float32)
    W2 = np.asarray(W2, dtype=np.float32)
    b2 = np.asarray(b2, dtype=np.float32)

    _, np_dt = _mm_dt()
    if PROFILE:
        _install_profile_shim()

    from concourse.bass_utils import run_bass_kernel_spmd

    xf = x.reshape(T, H)
    top2, w = _route(xf, Wg, bg)

    per_expert = _prep_weights(W1, b1, W2, b2, np_dt)

    # token lists per expert
    idx_list = []
    wgt_list = []
    for e in range(NUM_EXPERTS):
        mask = top2 == e  # [T, 2]
        idx = np.where(mask.any(axis=1))[0]
        slot = mask[idx, 1].astype(np.int64)  # 0 if slot0, 1 if slot1
        idx_list.append(idx)
        wgt_list.append(w[idx, slot])

    out = np.zeros((T, H), dtype=np.float32)
    max_count = max(len(i) for i in idx_list)
    # capacity: fit the hottest expert exactly (aligned), bounded so a
    # pathological distribution falls back to multiple rounds
    CAP = min(2048, max(512, -(-max_count // CAP_ALIGN) * CAP_ALIGN))
    if CAP not in _cache:
        _cache[CAP] = _build(CAP)
    nc = _cache[CAP]
    n_rounds = max(1, -(-max_count // CAP))

    for r in range(n_rounds):
        in_maps = []
        chunk_idx = []
        for e in range(NUM_EXPERTS):
            idx = idx_list[e][r * CAP : (r + 1) * CAP]
            chunk_idx.append(idx)
            c = len(idx)
            xe = np.zeros((H, CAP), dtype=np_dt)
            if c:
                xe[:, :c] = xf[idx].T.astype(np_dt)
            w1g, w2g, b1g, b2g = per_expert[e]
            in_maps.append(
                {
                    "xt_d": xe.reshape(KH, P, CAP),
                    "w1_d": w1g,
                    "b1_d": b1g,
                    "w2_d": w2g,
                    "b2_d": b2g,
                }
            )
        res = run_bass_kernel_spmd(
            nc,
            in_maps,
            core_ids=list(range(NUM_EXPERTS)),
            trace=bool(PROFILE),
        )
        if PROFILE:
            LAST_EXEC_NS = res.exec_time_ns
            LAST_RESULTS = res
        for e in range(NUM_EXPERTS):
            idx = chunk_idx[e]
            c = len(idx)
            if not c:
                continue
            yT = res.results[e]["y_d"].reshape(H, CAP)  # [H, CAP]
            we = wgt_list[e][r * CAP : (r + 1) * CAP]
            out[idx] += we[:, None] * np.asarray(yT[:, :c].T, dtype=np.float32)

    return out.reshape(B, S, H)
